# revision 20
# baseline (speedup 1.0000x reference)
"""CenterLoss kernel for Trainium2 (8 NeuronCores, data-parallel over batch).

reference:  mean(clip(rowsum((x - labels @ centers)^2), 1e-12, 1e12))
labels are exact one-hot rows, so labels @ centers is an embedding gather:
    idx[b]  = max_index(labels[b, :])           (DVE max_index, query = 1.0)
    c[b]    = centers[idx[b], :]                (indirect DMA row gather)
    ps[b]   = rowsum((x[b] - c[b])^2)           (DVE sub -> bf16, ACT square+accum)
Per-core output is a [128, 9] tile of per-sample sums (tile 7 split in two
halves for a shorter tail); the host merges the halves, applies the clip
(never binding for this data, but exact) and takes the mean.

Phase schedule (SWDGE + HWDGE queues running concurrently drop aggregate
DMA from ~430 to ~320 GB/s, so the phases are serialized):
  1. labels on the sync HWDGE ring (3MB, lands ~14.5us)
  2. index chain (DVE FIND_INDEX8) + row gathers on the SWDGE queue
     (8MB, solo, lands ~33us)
  3. x on the sync ring (8MB, solo), gated behind the last gather by a
     WAW write into xbig; 1MB chunks so sub/ACT pipeline per tile
  4. sub -> square+accum trail the x chunks; out store on the scalar
     ring right after the last accumulator read (same-engine ordering)
"""

import numpy as np

import concourse.bacc as bacc
import concourse.bass as bass
import concourse.mybir as mybir
from concourse.tile import TileContext
from concourse.bass_utils import run_bass_kernel_spmd

F32 = mybir.dt.float32
BF16 = mybir.dt.bfloat16
U32 = mybir.dt.uint32

NCORES = 8
B = 8192          # full batch
C = 751           # num classes
D = 2048          # feature dim
BS = B // NCORES  # batch per core = 1024
P = 128           # partitions
NT = BS // P      # batch tiles per core = 8
NQ = 4            # last tile split into quarters for a short tail
NACC = NT - 1 + NQ  # tile 7 contributes four quarter-sums

CLIP_LO, CLIP_HI = 1e-12, 1e12


def build_nc():
    nc = bacc.Bacc(
        "TRN2",
        target_bir_lowering=False,
        debug=False,
        num_devices=NCORES,
    )
    x = nc.dram_tensor("x", [BS, D], F32, kind="ExternalInput")
    labels = nc.dram_tensor("labels", [BS, C], F32, kind="ExternalInput")
    centers = nc.dram_tensor("centers", [C, D], F32, kind="ExternalInput")
    out = nc.dram_tensor("out", [P, NACC], F32, kind="ExternalOutput")

    with TileContext(nc) as tc:
        with tc.tile_pool(name="big", bufs=1) as bigpool:
            ones = bigpool.tile([P, 8], F32)
            idxs = bigpool.tile([P, NT, 8], U32)
            acc = bigpool.tile([P, NACC], F32)
            dif_a = bigpool.tile([P, D], BF16)
            dif_b = bigpool.tile([P, D], BF16)
            dsq = bigpool.tile([P, D], BF16)
            lbig = bigpool.tile([P, NT, C], F32)
            xbig = bigpool.tile([P, NT, D], F32)
            ctile = bigpool.tile([P, NT, D], F32)

            nc.vector.memset(ones[:], 1.0)

            labels_r = labels.rearrange("(n p) c -> p n c", p=P)
            x_r = x.rearrange("(n p) d -> p n d", p=P)

            # labels first on the SWDGE queue (ready at t0 -> scheduled
            # first). First chunk is a single tile so FIND0 (and with it the
            # first gather) unblocks ~2us earlier; the gather descriptors
            # then enter the queue before labels finish draining -> no gap.
            for lo, hi in ((0, 1), (1, 4), (4, 8)):
                nc.gpsimd.dma_start(
                    out=lbig[:, lo:hi, :], in_=labels_r[:, lo:hi, :]
                )

            Q = D // NQ

            def load_x(n):
                if n < NT - 1:
                    nc.gpsimd.dma_start(out=xbig[:, n, :], in_=x_r[:, n, :])
                else:
                    for q in range(NQ):
                        sl = slice(q * Q, (q + 1) * Q)
                        nc.gpsimd.dma_start(
                            out=xbig[:, n, sl], in_=x_r[:, n, sl]
                        )

            def gather(n):
                nc.gpsimd.indirect_dma_start(
                    out=ctile[:, n, :],
                    out_offset=None,
                    in_=centers[:],
                    in_offset=bass.IndirectOffsetOnAxis(
                        ap=idxs[:, n, 0:1], axis=0
                    ),
                )

            # FIND_n + gather_n interleaved (matches the static schedule that
            # benched fastest), then the x chunks; all loads ride the single
            # SWDGE queue FIFO so there is no cross-queue mixing penalty.
            for n in range(NT):
                nc.vector.max_index(
                    out=idxs[:, n, :], in_max=ones[:], in_values=lbig[:, n, :]
                )
                gather(n)
            for n in range(NT):
                load_x(n)

            # sub -> square+accum per tile, trailing the loads
            for n in range(NT - 1):
                dif = dif_a if n % 2 == 0 else dif_b
                nc.vector.tensor_sub(
                    out=dif[:], in0=xbig[:, n, :], in1=ctile[:, n, :]
                )
                nc.scalar.activation(
                    out=dsq[:],
                    in_=dif[:],
                    func=mybir.ActivationFunctionType.Square,
                    accum_out=acc[:, n:n + 1],
                )
            for q in range(NQ):
                sl = slice(q * Q, (q + 1) * Q)
                dif = dif_b if q % 2 == 0 else dif_a
                nc.vector.tensor_sub(
                    out=dif[:, sl], in0=xbig[:, NT - 1, sl], in1=ctile[:, NT - 1, sl]
                )
                nc.scalar.activation(
                    out=dsq[:, sl],
                    in_=dif[:, sl],
                    func=mybir.ActivationFunctionType.Square,
                    accum_out=acc[:, NT - 1 + q:NT + q],
                )

            # out store on the warm SWDGE queue (cold HWDGE rings cost ~4us
            # on first use)
            nc.gpsimd.dma_start(out=out[:], in_=acc[:])

    nc.compile()
    return nc


_NC = None


def _get_nc():
    global _NC
    if _NC is None:
        _NC = build_nc()
    return _NC


def run_sharded(inputs: dict, trace: bool = False):
    """Shard, run on 8 cores, return (per_sample [B] f32, BassKernelResults)."""
    x = np.ascontiguousarray(np.asarray(inputs["x"], dtype=np.float32))
    labels = np.ascontiguousarray(np.asarray(inputs["labels"], dtype=np.float32))
    centers = np.ascontiguousarray(np.asarray(inputs["centers"], dtype=np.float32))
    assert x.shape == (B, D) and labels.shape == (B, C) and centers.shape == (C, D)

    in_maps = [
        {
            "x": np.ascontiguousarray(x[k * BS:(k + 1) * BS]),
            "labels": np.ascontiguousarray(labels[k * BS:(k + 1) * BS]),
            "centers": centers,
        }
        for k in range(NCORES)
    ]
    res = run_bass_kernel_spmd(
        _get_nc(), in_maps, core_ids=list(range(NCORES)), trace=trace
    )
    # out[p, n] holds sample k*BS + n*P + p; cols NT-1.. are the NQ
    # quarter-sums of the last tile
    def merge(o):
        last = o[:, NT - 1:].sum(axis=1, keepdims=True)
        return np.concatenate([o[:, :NT - 1], last], axis=1)

    per_sample = np.concatenate(
        [merge(res.results[k]["out"]).T.reshape(-1) for k in range(NCORES)]
    )
    return per_sample, res


def kernel(x, labels, centers):
    per_sample, _ = run_sharded({"x": x, "labels": labels, "centers": centers})
    per_sample = np.clip(per_sample, CLIP_LO, CLIP_HI)
    return np.asarray(per_sample.mean(dtype=np.float64), dtype=np.float32)


# revision 21
# speedup vs baseline: 1.1262x; 1.1262x over previous
"""CenterLoss kernel for Trainium2 (8 NeuronCores, data-parallel over batch).

reference:  mean(clip(rowsum((x - labels @ centers)^2), 1e-12, 1e12))
labels are exact one-hot rows, so labels @ centers is an embedding gather:
    idx[b]  = max_index(labels[b, :])           (DVE max_index, query = 1.0)
    c[b]    = centers[idx[b], :]                (indirect DMA row gather)
    ps[b]   = rowsum((x[b] - c[b])^2)           (DVE sub, ACT square+f32 accum)

All three input streams are cast to bf16 at shard time (the tolerance for
this loss is 2e-2; bf16 keeps the scalar error ~1e-4), halving HBM traffic
to ~9.6MB/core. One-hot labels are exact in bf16, per-sample sums
accumulate in f32 on the ACT engine.

Schedule: every load rides the single SWDGE queue FIFO (two queues running
concurrently drop aggregate DMA from ~430 to ~320 GB/s, so one queue only):
labels (split so FIND0 unblocks early) -> row gathers as the index chain
delivers offsets -> x chunks, with the last tile split into quarters so the
sub/square tail after the final chunk is short. Per-core output is a
[128, 11] tile of per-sample (partial) sums; the host merges the last
tile's quarters, applies the clip (never binding for this data, but exact)
and takes the mean.
"""

import numpy as np
import ml_dtypes

import concourse.bacc as bacc
import concourse.bass as bass
import concourse.mybir as mybir
from concourse.tile import TileContext
from concourse.bass_utils import run_bass_kernel_spmd

F32 = mybir.dt.float32
BF16 = mybir.dt.bfloat16
U32 = mybir.dt.uint32
NP_BF16 = ml_dtypes.bfloat16

NCORES = 8
B = 8192          # full batch
C = 751           # num classes
D = 2048          # feature dim
BS = B // NCORES  # batch per core = 1024
P = 128           # partitions
NT = BS // P      # batch tiles per core = 8
NQ = 4            # last tile split into quarters for a short tail
NACC = NT - 1 + NQ

CLIP_LO, CLIP_HI = 1e-12, 1e12


def build_nc():
    nc = bacc.Bacc(
        "TRN2",
        target_bir_lowering=False,
        debug=False,
        num_devices=NCORES,
    )
    x = nc.dram_tensor("x", [BS, D], BF16, kind="ExternalInput")
    labels = nc.dram_tensor("labels", [BS, C], BF16, kind="ExternalInput")
    centers = nc.dram_tensor("centers", [C, D], BF16, kind="ExternalInput")
    out = nc.dram_tensor("out", [P, NACC], F32, kind="ExternalOutput")

    with TileContext(nc) as tc:
        with tc.tile_pool(name="big", bufs=1) as pool:
            ones = pool.tile([P, 8], BF16)
            idxs = pool.tile([P, NT, 8], U32)
            acc = pool.tile([P, NACC], F32)
            dif_a = pool.tile([P, D], BF16)
            dif_b = pool.tile([P, D], BF16)
            dsq = pool.tile([P, D], BF16)
            lbig = pool.tile([P, NT, C], BF16)
            xbig = pool.tile([P, NT, D], BF16)
            ctile = pool.tile([P, NT, D], BF16)

            nc.vector.memset(ones[:], 1.0)

            labels_r = labels.rearrange("(n p) c -> p n c", p=P)
            x_r = x.rearrange("(n p) d -> p n d", p=P)

            # labels first on the SWDGE queue; first chunk is one tile so
            # FIND0 (and the first gather) unblocks early
            for lo, hi in ((0, 1), (1, 4), (4, 8)):
                nc.gpsimd.dma_start(
                    out=lbig[:, lo:hi, :], in_=labels_r[:, lo:hi, :]
                )

            # FIND_n + gather_n interleaved; gathers slot into the queue as
            # the index chain delivers offsets
            for n in range(NT):
                nc.vector.max_index(
                    out=idxs[:, n, :], in_max=ones[:], in_values=lbig[:, n, :]
                )
                nc.gpsimd.indirect_dma_start(
                    out=ctile[:, n, :],
                    out_offset=None,
                    in_=centers[:],
                    in_offset=bass.IndirectOffsetOnAxis(
                        ap=idxs[:, n, 0:1], axis=0
                    ),
                )

            # x chunks last; final tile in quarters for a short tail
            Q = D // NQ
            for n in range(NT - 1):
                nc.gpsimd.dma_start(out=xbig[:, n, :], in_=x_r[:, n, :])
            for q in range(NQ):
                sl = slice(q * Q, (q + 1) * Q)
                nc.gpsimd.dma_start(
                    out=xbig[:, NT - 1, sl], in_=x_r[:, NT - 1, sl]
                )

            # sub -> square + f32 accum per tile, trailing the x chunks
            for n in range(NT - 1):
                dif = dif_a if n % 2 == 0 else dif_b
                nc.vector.tensor_sub(
                    out=dif[:], in0=xbig[:, n, :], in1=ctile[:, n, :]
                )
                nc.scalar.activation(
                    out=dsq[:],
                    in_=dif[:],
                    func=mybir.ActivationFunctionType.Square,
                    accum_out=acc[:, n:n + 1],
                )
            for q in range(NQ):
                sl = slice(q * Q, (q + 1) * Q)
                dif = dif_b if q % 2 == 0 else dif_a
                nc.vector.tensor_sub(
                    out=dif[:, sl], in0=xbig[:, NT - 1, sl], in1=ctile[:, NT - 1, sl]
                )
                nc.scalar.activation(
                    out=dsq[:, sl],
                    in_=dif[:, sl],
                    func=mybir.ActivationFunctionType.Square,
                    accum_out=acc[:, NT - 1 + q:NT + q],
                )

            # out store on the warm SWDGE queue
            nc.gpsimd.dma_start(out=out[:], in_=acc[:])

    nc.compile()
    return nc


_NC = None


def _get_nc():
    global _NC
    if _NC is None:
        _NC = build_nc()
    return _NC


def _shard(inputs: dict):
    x = np.asarray(inputs["x"]).astype(NP_BF16)
    labels = np.asarray(inputs["labels"]).astype(NP_BF16)
    centers = np.ascontiguousarray(np.asarray(inputs["centers"]).astype(NP_BF16))
    assert x.shape == (B, D) and labels.shape == (B, C) and centers.shape == (C, D)
    return [
        {
            "x": np.ascontiguousarray(x[k * BS:(k + 1) * BS]),
            "labels": np.ascontiguousarray(labels[k * BS:(k + 1) * BS]),
            "centers": centers,
        }
        for k in range(NCORES)
    ]


def run_sharded(inputs: dict, trace: bool = False):
    """Shard, run on 8 cores, return (per_sample [B] f32, BassKernelResults)."""
    in_maps = _shard(inputs)
    res = run_bass_kernel_spmd(
        _get_nc(), in_maps, core_ids=list(range(NCORES)), trace=trace
    )
    # out[p, n] holds sample k*BS + n*P + p; cols NT-1.. are the NQ
    # quarter-sums of the last tile
    def merge(o):
        last = o[:, NT - 1:].sum(axis=1, keepdims=True)
        return np.concatenate([o[:, :NT - 1], last], axis=1)

    per_sample = np.concatenate(
        [merge(res.results[k]["out"]).T.reshape(-1) for k in range(NCORES)]
    )
    return per_sample, res


def kernel(x, labels, centers):
    per_sample, _ = run_sharded({"x": x, "labels": labels, "centers": centers})
    per_sample = np.clip(per_sample, CLIP_LO, CLIP_HI)
    return np.asarray(per_sample.mean(dtype=np.float64), dtype=np.float32)


# revision 25
# speedup vs baseline: 1.1290x; 1.0025x over previous
"""CenterLoss kernel for Trainium2 (8 NeuronCores, data-parallel over batch).

reference:  mean(clip(rowsum((x - labels @ centers)^2), 1e-12, 1e12))
labels are exact one-hot rows, so labels @ centers is an embedding gather:
    idx[b]  = max_index(labels[b, :])           (DVE max_index, query = 1.0)
    c[b]    = centers[idx[b], :]                (indirect DMA row gather)
    ps[b]   = rowsum((x[b] - c[b])^2)           (DVE sub, ACT square+f32 accum)

All three input streams are cast to bf16 at shard time (the tolerance for
this loss is 2e-2; bf16 keeps the scalar error ~1e-4), halving HBM traffic
to ~9.6MB/core. One-hot labels are exact in bf16, per-sample sums
accumulate in f32 on the ACT engine.

Schedule: every load rides the single SWDGE queue FIFO (two queues running
concurrently drop aggregate DMA from ~430 to ~320 GB/s, so one queue only):
labels (split so FIND0 unblocks early) -> row gathers as the index chain
delivers offsets -> x chunks, with the last tile split into quarters so the
sub/square tail after the final chunk is short. Per-core output is a
[128, 11] tile of per-sample (partial) sums; the host merges the last
tile's quarters, applies the clip (never binding for this data, but exact)
and takes the mean.
"""

import numpy as np
import ml_dtypes

import concourse.bacc as bacc
import concourse.bass as bass
import concourse.mybir as mybir
from concourse.tile import TileContext
from concourse.bass_utils import run_bass_kernel_spmd

F32 = mybir.dt.float32
BF16 = mybir.dt.bfloat16
U32 = mybir.dt.uint32
NP_BF16 = ml_dtypes.bfloat16

NCORES = 8
B = 8192          # full batch
C = 751           # num classes
D = 2048          # feature dim
BS = B // NCORES  # batch per core = 1024
P = 128           # partitions
NT = BS // P      # batch tiles per core = 8
NQ = 4            # last tile split into quarters for a short tail
NACC = NT - 1 + NQ

CLIP_LO, CLIP_HI = 1e-12, 1e12


def build_nc():
    nc = bacc.Bacc(
        "TRN2",
        target_bir_lowering=False,
        debug=False,
        num_devices=NCORES,
    )
    x = nc.dram_tensor("x", [BS, D], BF16, kind="ExternalInput")
    labels = nc.dram_tensor("labels", [BS, C], BF16, kind="ExternalInput")
    centers = nc.dram_tensor("centers", [C, D], BF16, kind="ExternalInput")
    out = nc.dram_tensor("out", [P, NACC], F32, kind="ExternalOutput")

    with TileContext(nc) as tc:
        with tc.tile_pool(name="big", bufs=1) as pool:
            ones = pool.tile([P, 8], BF16)
            idxs = pool.tile([P, NT, 8], U32)
            acc = pool.tile([P, NACC], F32)
            dif_a = pool.tile([P, D], BF16)
            dif_b = pool.tile([P, D], BF16)
            dsq = pool.tile([P, D], BF16)
            tjunk = pool.tile([P, D], BF16)
            lbig = pool.tile([P, NT, C], BF16)
            xbig = pool.tile([P, NT, D], BF16)
            ctile = pool.tile([P, NT, D], BF16)

            nc.vector.memset(ones[:], 1.0)

            labels_r = labels.rearrange("(n p) c -> p n c", p=P)
            x_r = x.rearrange("(n p) d -> p n d", p=P)

            # labels first on the SWDGE queue; first chunk is one tile so
            # FIND0 (and the first gather) unblocks early
            for lo, hi in ((0, 1), (1, 4), (4, 8)):
                nc.gpsimd.dma_start(
                    out=lbig[:, lo:hi, :], in_=labels_r[:, lo:hi, :]
                )

            # FIND_n + gather_n interleaved; gathers slot into the queue as
            # the index chain delivers offsets
            for n in range(NT):
                nc.vector.max_index(
                    out=idxs[:, n, :], in_max=ones[:], in_values=lbig[:, n, :]
                )
                nc.gpsimd.indirect_dma_start(
                    out=ctile[:, n, :],
                    out_offset=None,
                    in_=centers[:],
                    in_offset=bass.IndirectOffsetOnAxis(
                        ap=idxs[:, n, 0:1], axis=0
                    ),
                )

            # x chunks last; final tile in quarters for a short tail
            Q = D // NQ
            for lo, hi in ((0, 2), (2, 4), (4, 6), (6, 7)):
                nc.gpsimd.dma_start(
                    out=xbig[:, lo:hi, :], in_=x_r[:, lo:hi, :]
                )
            for q in range(NQ):
                sl = slice(q * Q, (q + 1) * Q)
                nc.gpsimd.dma_start(
                    out=xbig[:, NT - 1, sl], in_=x_r[:, NT - 1, sl]
                )

            # sub on DVE, then square + f32 accum alternating between the
            # ACT engine (even tiles) and a DVE tensor_tensor_reduce (odd
            # tiles) so neither engine's ~2.4us/tile chain is the critical
            # path
            def square_accum(dif_ap, acc_col, on_act):
                if on_act:
                    nc.scalar.activation(
                        out=dsq[:, 0:dif_ap.shape[-1]],
                        in_=dif_ap,
                        func=mybir.ActivationFunctionType.Square,
                        accum_out=acc[:, acc_col:acc_col + 1],
                    )
                else:
                    nc.vector.tensor_tensor_reduce(
                        out=tjunk[:, 0:dif_ap.shape[-1]],
                        in0=dif_ap,
                        in1=dif_ap,
                        scale=1.0,
                        scalar=0.0,
                        op0=mybir.AluOpType.mult,
                        op1=mybir.AluOpType.add,
                        accum_out=acc[:, acc_col:acc_col + 1],
                    )

            for n in range(NT - 1):
                dif = dif_a if n % 2 == 0 else dif_b
                nc.vector.tensor_sub(
                    out=dif[:], in0=xbig[:, n, :], in1=ctile[:, n, :]
                )
                square_accum(dif[:], n, on_act=True)
            for q in range(NQ):
                sl = slice(q * Q, (q + 1) * Q)
                dif = dif_b if q % 2 == 0 else dif_a
                nc.vector.tensor_sub(
                    out=dif[:, sl], in0=xbig[:, NT - 1, sl], in1=ctile[:, NT - 1, sl]
                )
                square_accum(dif[:, sl], NT - 1 + q, on_act=True)

            # out store on the warm SWDGE queue
            nc.gpsimd.dma_start(out=out[:], in_=acc[:])

    nc.compile()
    return nc


_NC = None


def _get_nc():
    global _NC
    if _NC is None:
        _NC = build_nc()
    return _NC


def _shard(inputs: dict):
    x = np.asarray(inputs["x"]).astype(NP_BF16)
    labels = np.asarray(inputs["labels"]).astype(NP_BF16)
    centers = np.ascontiguousarray(np.asarray(inputs["centers"]).astype(NP_BF16))
    assert x.shape == (B, D) and labels.shape == (B, C) and centers.shape == (C, D)
    return [
        {
            "x": np.ascontiguousarray(x[k * BS:(k + 1) * BS]),
            "labels": np.ascontiguousarray(labels[k * BS:(k + 1) * BS]),
            "centers": centers,
        }
        for k in range(NCORES)
    ]


def run_sharded(inputs: dict, trace: bool = False):
    """Shard, run on 8 cores, return (per_sample [B] f32, BassKernelResults)."""
    in_maps = _shard(inputs)
    res = run_bass_kernel_spmd(
        _get_nc(), in_maps, core_ids=list(range(NCORES)), trace=trace
    )
    # out[p, n] holds sample k*BS + n*P + p; cols NT-1.. are the NQ
    # quarter-sums of the last tile
    def merge(o):
        last = o[:, NT - 1:].sum(axis=1, keepdims=True)
        return np.concatenate([o[:, :NT - 1], last], axis=1)

    per_sample = np.concatenate(
        [merge(res.results[k]["out"]).T.reshape(-1) for k in range(NCORES)]
    )
    return per_sample, res


def kernel(x, labels, centers):
    per_sample, _ = run_sharded({"x": x, "labels": labels, "centers": centers})
    per_sample = np.clip(per_sample, CLIP_LO, CLIP_HI)
    return np.asarray(per_sample.mean(dtype=np.float64), dtype=np.float32)


# revision 27
# speedup vs baseline: 1.4300x; 1.2666x over previous
"""CenterLoss kernel for Trainium2 (8 NeuronCores, data-parallel over batch).

reference:  mean(clip(rowsum((x - labels @ centers)^2), 1e-12, 1e12))
labels are exact one-hot rows, so labels @ centers is an embedding gather:
    idx[b]  = max_index(labels[b, :])           (DVE max_index, query = 1.0)
    c[b]    = centers[idx[b], :]                (indirect DMA row gather)
    ps[b]   = rowsum((x[b] - c[b])^2)           (DVE sub, ACT square+f32 accum)

All three input streams are cast to bf16 at shard time (the tolerance for
this loss is 2e-2; bf16 keeps the scalar error ~1e-4), halving HBM traffic
to ~9.6MB/core. One-hot labels are exact in bf16, per-sample sums
accumulate in f32 on the ACT engine.

Schedule: every load rides the single SWDGE queue FIFO (two queues running
concurrently drop aggregate DMA from ~430 to ~320 GB/s, so one queue only):
labels (split so FIND0 unblocks early) -> row gathers as the index chain
delivers offsets -> x chunks, with the last tile split into quarters so the
sub/square tail after the final chunk is short. Per-core output is a
[128, 11] tile of per-sample (partial) sums; the host merges the last
tile's quarters, applies the clip (never binding for this data, but exact)
and takes the mean.
"""

import numpy as np
import ml_dtypes

import concourse.bacc as bacc
import concourse.bass as bass
import concourse.mybir as mybir
from concourse.tile import TileContext
from concourse.bass_utils import run_bass_kernel_spmd

F32 = mybir.dt.float32
BF16 = mybir.dt.bfloat16
U32 = mybir.dt.uint32
NP_BF16 = ml_dtypes.bfloat16

NCORES = 8
B = 8192          # full batch
C = 751           # num classes
D = 2048          # feature dim
BS = B // NCORES  # batch per core = 1024
P = 128           # partitions
NT = BS // P      # batch tiles per core = 8
NQ = 4            # last tile split into quarters for a short tail
NACC = NT - 1 + NQ

CLIP_LO, CLIP_HI = 1e-12, 1e12


def build_nc():
    nc = bacc.Bacc(
        "TRN2",
        target_bir_lowering=False,
        debug=False,
        num_devices=NCORES,
    )
    x = nc.dram_tensor("x", [BS, D], BF16, kind="ExternalInput")
    labels = nc.dram_tensor("labels", [BS, C], BF16, kind="ExternalInput")
    centers = nc.dram_tensor("centers", [C, D], BF16, kind="ExternalInput")
    out = nc.dram_tensor("out", [P, NACC], F32, kind="ExternalOutput")

    with TileContext(nc) as tc:
        with tc.tile_pool(name="big", bufs=1) as pool:
            ones = pool.tile([P, 8], BF16)
            idxs = pool.tile([P, NT, 8], U32)
            acc = pool.tile([P, NACC], F32)
            dif_a = pool.tile([P, D], BF16)
            dif_b = pool.tile([P, D], BF16)
            dsq = pool.tile([P, D], BF16)
            tjunk = pool.tile([P, D], BF16)
            lbig = pool.tile([P, NT, C], BF16)
            xbig = pool.tile([P, NT, D], BF16)
            ctile = pool.tile([P, NT, D], BF16)

            nc.vector.memset(ones[:], 1.0)

            labels_r = labels.rearrange("(n p) c -> p n c", p=P)
            x_r = x.rearrange("(n p) d -> p n d", p=P)

            # labels first on the SWDGE queue; first chunk is one tile so
            # FIND0 (and the first gather) unblocks early
            for lo, hi in ((0, 1), (1, 4), (4, 8)):
                nc.gpsimd.dma_start(
                    out=lbig[:, lo:hi, :], in_=labels_r[:, lo:hi, :]
                )

            # FIND_n + gather_n interleaved, with x chunks emitted between
            # gathers so the queue always has ready work while the index
            # chain paces the gathers; final x tile in quarters for a short
            # tail
            Q = D // NQ
            x_chunks = [("full", lo, hi) for lo, hi in ((0, 2), (2, 4), (4, 6), (6, 7))]
            x_chunks += [("quarter", q * Q, (q + 1) * Q) for q in range(NQ)]

            def load_x(i):
                kind, lo, hi = x_chunks[i]
                if kind == "full":
                    nc.gpsimd.dma_start(
                        out=xbig[:, lo:hi, :], in_=x_r[:, lo:hi, :]
                    )
                else:
                    nc.gpsimd.dma_start(
                        out=xbig[:, NT - 1, lo:hi], in_=x_r[:, NT - 1, lo:hi]
                    )

            for n in range(NT):
                nc.vector.max_index(
                    out=idxs[:, n, :], in_max=ones[:], in_values=lbig[:, n, :]
                )
                nc.gpsimd.indirect_dma_start(
                    out=ctile[:, n, :],
                    out_offset=None,
                    in_=centers[:],
                    in_offset=bass.IndirectOffsetOnAxis(
                        ap=idxs[:, n, 0:1], axis=0
                    ),
                )
                if n >= 1 and n - 1 < len(x_chunks):
                    load_x(n - 1)
            for i in range(NT - 1, len(x_chunks)):
                load_x(i)

            # sub on DVE, then square + f32 accum. The ACT engine is capped
            # at 1 elem/cycle (2.7us per tile incl accumulator read), so a
            # few tiles instead square on DVE (tensor mult + reduce_sum,
            # ~2.1us) to balance the two chains at ~17us each.
            DVE_SQ_FULL = {2, 5}
            DVE_SQ_QUARTER = {1}

            def square_accum(dif_ap, width, acc_col, on_act):
                if on_act:
                    nc.scalar.activation(
                        out=dsq[:, 0:width],
                        in_=dif_ap,
                        func=mybir.ActivationFunctionType.Square,
                        accum_out=acc[:, acc_col:acc_col + 1],
                    )
                else:
                    nc.vector.tensor_mul(
                        out=tjunk[:, 0:width], in0=dif_ap, in1=dif_ap
                    )
                    nc.vector.reduce_sum(
                        out=acc[:, acc_col:acc_col + 1],
                        in_=tjunk[:, 0:width],
                        axis=mybir.AxisListType.X,
                    )

            for n in range(NT - 1):
                dif = dif_a if n % 2 == 0 else dif_b
                nc.vector.tensor_sub(
                    out=dif[:], in0=xbig[:, n, :], in1=ctile[:, n, :]
                )
                square_accum(dif[:], D, n, on_act=(n not in DVE_SQ_FULL))
            for q in range(NQ):
                sl = slice(q * Q, (q + 1) * Q)
                dif = dif_b if q % 2 == 0 else dif_a
                nc.vector.tensor_sub(
                    out=dif[:, sl], in0=xbig[:, NT - 1, sl], in1=ctile[:, NT - 1, sl]
                )
                square_accum(
                    dif[:, sl], Q, NT - 1 + q, on_act=(q not in DVE_SQ_QUARTER)
                )

            # out store on the warm SWDGE queue
            nc.gpsimd.dma_start(out=out[:], in_=acc[:])

    nc.compile()
    return nc


_NC = None


def _get_nc():
    global _NC
    if _NC is None:
        _NC = build_nc()
    return _NC


def _shard(inputs: dict):
    x = np.asarray(inputs["x"]).astype(NP_BF16)
    labels = np.asarray(inputs["labels"]).astype(NP_BF16)
    centers = np.ascontiguousarray(np.asarray(inputs["centers"]).astype(NP_BF16))
    assert x.shape == (B, D) and labels.shape == (B, C) and centers.shape == (C, D)
    return [
        {
            "x": np.ascontiguousarray(x[k * BS:(k + 1) * BS]),
            "labels": np.ascontiguousarray(labels[k * BS:(k + 1) * BS]),
            "centers": centers,
        }
        for k in range(NCORES)
    ]


def run_sharded(inputs: dict, trace: bool = False):
    """Shard, run on 8 cores, return (per_sample [B] f32, BassKernelResults)."""
    in_maps = _shard(inputs)
    res = run_bass_kernel_spmd(
        _get_nc(), in_maps, core_ids=list(range(NCORES)), trace=trace
    )
    # out[p, n] holds sample k*BS + n*P + p; cols NT-1.. are the NQ
    # quarter-sums of the last tile
    def merge(o):
        last = o[:, NT - 1:].sum(axis=1, keepdims=True)
        return np.concatenate([o[:, :NT - 1], last], axis=1)

    per_sample = np.concatenate(
        [merge(res.results[k]["out"]).T.reshape(-1) for k in range(NCORES)]
    )
    return per_sample, res


def kernel(x, labels, centers):
    per_sample, _ = run_sharded({"x": x, "labels": labels, "centers": centers})
    per_sample = np.clip(per_sample, CLIP_LO, CLIP_HI)
    return np.asarray(per_sample.mean(dtype=np.float64), dtype=np.float32)
